# revision 24
# baseline (speedup 1.0000x reference)
"""Median graph convolution on 8 Trainium2 NeuronCores.

out[n, c] = median over valid neighbors j of (x @ kernel)[neighbors[n, j], c]
(lower median, rank (deg-1)//2 of the first deg neighbor slots).

Strategy (data-parallel over nodes, 6272 nodes/core):
  - host sorts nodes by degree (descending), striped across the 8 cores so
    every core sees the same degree profile and one compiled program fits all
  - each core matmuls its node shard on the PE -> h shard (fp16),
    AllGather into a per-core HBM table with trailing +inf sentinel rows
  - the table is indexed as 512-byte PAIR rows (two h rows per descriptor),
    so the 50176-row table needs only 25089 int16-indexable pair rows;
    each real neighbor costs exactly ONE gather descriptor
  - only the first maxdeg(tile) slots are gathered per 128-node tile
    (pads ride as +inf sentinel descriptors / vector memset)
  - a copy + copy_predicated (int16 parity mask, stride-0 broadcast over
    channels) selects the wanted half of each gathered pair
  - a degree-adaptive bitonic network sorts the two H-halves of the slot
    array and a rank-r two-way merge formula extracts the lower median
"""

import sys

sys.path.insert(0, "/opt/trn_rl_repo")

import numpy as np

N, K, IN_C, OUT_C = 50000, 32, 256, 128
NCORES = 8
NTILES = 49                      # 128-node tiles per core
SHARD = NTILES * 128             # 6272
NPAD = SHARD * NCORES            # 50176
TROWS = NPAD + 4                 # +inf sentinel rows at the end
SENT_PAIR = NPAD // 2            # pair index of the +inf sentinel row pair
NPAIRS = SENT_PAIR + 1           # pair rows addressable by the gather
GCHUNK = 8                       # slots per dma_gather call (8*128 = 1024 idx)
MAXSLOTS = 32

_CACHE = {}


def _next_pow2(x):
    p = 1
    while p < x:
        p *= 2
    return p


def _make_schedule(deg_sorted):
    """Per-tile (maxd, H, r_list) from the global descending degree profile."""
    sched = []
    for t in range(NTILES):
        degs = deg_sorted[t * 128 * NCORES:(t + 1) * 128 * NCORES]
        maxd = int(degs[0])
        H = max(1, _next_pow2(maxd) // 2)
        rs = sorted({int((d - 1) // 2) for d in degs}, reverse=True)
        sched.append((maxd, H, tuple(rs)))
    return tuple(sched)


def _emit_program(sched):
    import concourse.tile as tile
    import concourse.mybir as mybir
    from concourse import bacc
    from concourse.bass import AP
    from concourse.library_config import mlp

    fp16 = mybir.dt.float16
    fp32 = mybir.dt.float32
    i16 = mybir.dt.int16
    Alu = mybir.AluOpType

    tot_idx_cols = sum(maxd * 8 for (maxd, _, _) in sched)
    tot_par_cols = sum(maxd for (maxd, _, _) in sched)
    tot_pick = sum(len(rs) - 1 for (_, _, rs) in sched)

    nc = bacc.Bacc("TRN2", target_bir_lowering=False, num_swdge_queues=4,
                   dynamic_dma_scratch_size=32768)

    xT = nc.dram_tensor("xT", [IN_C, SHARD], fp16, kind="ExternalInput")
    w = nc.dram_tensor("w", [IN_C, OUT_C], fp16, kind="ExternalInput")
    idx_d = nc.dram_tensor("idx", [128, tot_idx_cols], i16, kind="ExternalInput")
    par_d = nc.dram_tensor("par", [128, tot_par_cols], i16, kind="ExternalInput")
    pick_d = nc.dram_tensor("pick", [128, max(1, tot_pick)], i16, kind="ExternalInput")
    infs = nc.dram_tensor("infs", [4, OUT_C], fp16, kind="ExternalInput")  # +inf rows
    out = nc.dram_tensor("out", [SHARD, OUT_C], fp32, kind="ExternalOutput")
    table = nc.dram_tensor("table", [TROWS, OUT_C], fp16, addr_space="Shared")
    hshard = nc.dram_tensor("hshard", [SHARD, OUT_C], fp16)

    # gather source: the table viewed as 512B pair rows [NPAIRS, 256]
    pair_ap = AP(table[:].tensor, 0, [[2 * OUT_C, NPAIRS], [1, 2 * OUT_C]])

    S = OUT_C  # slot stride (elements) in the selected-value tile v

    def slot_ap(t, slot0, dims, stride=None):
        """AP over value tile t: partition dim + (slot_step, count) dims + c.

        stride overrides the slot stride in elements (256 for the raw pair
        buffer whose a-halves act as the stage-0 value array)."""
        ss = S if stride is None else stride
        base = t[:]
        free = [[st * ss, ct] for (st, ct) in dims if ct != 1]
        return AP(base.tensor, base.offset + slot0 * ss, [base.ap[0]] + free + [[1, OUT_C]])

    def stages_for(top):
        """(k, j, allasc) stage list; allasc on the final k group."""
        ks = []
        k = 2
        while k <= top:
            j = k // 2
            while j >= 1:
                ks.append((k, j, k == top))
                j //= 2
            k *= 2
        return ks

    with tile.TileContext(nc) as tc:
        nc.gpsimd.load_library(mlp)
        with (
            tc.tile_pool(name="const", bufs=1) as cpool,
            tc.tile_pool(name="psum", bufs=2, space="PSUM") as psum_pool,
            tc.tile_pool(name="gbuf", bufs=4) as gpool,
            tc.tile_pool(name="work", bufs=2) as wpool,
            tc.tile_pool(name="mout", bufs=2) as mpool,
        ):
            # ---- phase 1+2: h rows = x @ w (x chunk stationary -> [node, c]),
            # AllGather pipelined in chunks behind the matmul ----
            inft = cpool.tile([4, OUT_C], fp16)
            nc.sync.dma_start(inft[:], infs[:])
            nc.sync.dma_start(table[NPAD:NPAD + 4, :], inft[:])
            AGCH = 7                    # tiles per pipelined AllGather chunk
            with tc.tile_pool(name="stage", bufs=1) as spool:
                lw0 = spool.tile([128, OUT_C], fp16)
                lw1 = spool.tile([128, OUT_C], fp16)
                nc.sync.dma_start(lw0[:], w[0:128, :])
                nc.sync.dma_start(lw1[:], w[128:256, :])
                xt0 = spool.tile([128, SHARD], fp16)
                xt1 = spool.tile([128, SHARD], fp16)
                nc.sync.dma_start(xt0[:], xT[0:128, :])
                nc.sync.dma_start(xt1[:], xT[128:256, :])
                hrows = spool.tile([128, NTILES, OUT_C], fp16)
                tb = table[:]
                for j in range(NTILES):
                    ns = slice(j * 128, (j + 1) * 128)
                    ps = psum_pool.tile([128, OUT_C], fp32)
                    nc.tensor.matmul(ps[:], lhsT=xt0[:, ns], rhs=lw0[:], start=True, stop=False)
                    nc.tensor.matmul(ps[:], lhsT=xt1[:, ns], rhs=lw1[:], start=False, stop=True)
                    nc.scalar.copy(hrows[:, j, :], ps[:])
                    if (j + 1) % AGCH == 0:
                        c0 = j + 1 - AGCH
                        rows = AGCH * 128
                        nc.sync.dma_start(
                            hshard[c0 * 128:(j + 1) * 128, :].rearrange(
                                "(j n) c -> n j c", n=128),
                            hrows[:, c0:j + 1, :],
                        )
                        nc.gpsimd.collective_compute(
                            "AllGather",
                            mybir.AluOpType.bypass,
                            replica_groups=[list(range(NCORES))],
                            ins=[hshard[c0 * 128:(j + 1) * 128, :]],
                            outs=[table[c0 * 128 * NCORES:(j + 1) * 128 * NCORES, :]],
                        )

            # ---- load index/mask streams; +inf constant for pad slots ----
            idx_sb = cpool.tile([128, tot_idx_cols], i16)
            par_sb = cpool.tile([128, tot_par_cols], i16)
            pick_sb = cpool.tile([128, max(1, tot_pick)], i16)
            nc.sync.dma_start(idx_sb[:], idx_d[:])
            nc.sync.dma_start(par_sb[:], par_d[:])
            nc.sync.dma_start(pick_sb[:], pick_d[:])
            inf_const = cpool.tile([128, 15 * OUT_C], fp16)
            nc.vector.memset(inf_const[:], float("inf"))

            # ---- phase 3: gather + select + sort + median per tile ----
            icol = 0      # running idx column offset
            pcol = 0      # running parity column offset
            kcol = 0      # running pick-mask column offset
            qn = 0        # dma queue rotation
            for t, (maxd, H, rs) in enumerate(sched):
                P2 = 2 * H
                buf = gpool.tile([128, MAXSLOTS, 2 * OUT_C], fp16, tag="pair")
                for s0 in range(0, maxd, GCHUNK):
                    s1 = min(s0 + GCHUNK, maxd)
                    G = (s1 - s0) * 128
                    nc.gpsimd.dma_gather(
                        buf[:, s0:s1, :],
                        pair_ap,
                        idx_sb[:, icol + s0 * 8: icol + s1 * 8],
                        G, G, 2 * OUT_C,
                        queue_num=qn, single_packet=False)
                    qn = (qn + 1) % 4
                icol += maxd * 8

                # select the wanted half of each pair into v0: copy the
                # a-halves, overwrite with the b-half where the parity mask
                # is 1 (stride-0 broadcast mask), memset pad slots to +inf.
                SB = 2 * OUT_C
                bb = buf[:]
                a_ap = AP(bb.tensor, bb.offset, [bb.ap[0], [SB, maxd], [1, OUT_C]])
                b_ap = AP(bb.tensor, bb.offset + OUT_C, [bb.ap[0], [SB, maxd], [1, OUT_C]])
                pp = par_sb[:]
                m_ap = AP(pp.tensor, pp.offset + pcol, [pp.ap[0], [1, maxd], [0, OUT_C]])
                pcol += maxd

                v0 = wpool.tile([128, MAXSLOTS, OUT_C], fp16, tag="v0")
                v1 = wpool.tile([128, MAXSLOTS, OUT_C], fp16, tag="v1")
                nc.vector.tensor_copy(slot_ap(v0, 0, [(1, maxd)]), a_ap)
                if maxd < P2:
                    nc.gpsimd.memset(slot_ap(v0, maxd, [(1, P2 - maxd)]), float("inf"))
                nc.vector.copy_predicated(slot_ap(v0, 0, [(1, maxd)]), m_ap, b_ap)

                def emit_net(stages, base, W, cur, other):
                    """Bitonic network over slots [base, base+W); returns the
                    tile holding the final values of that region."""
                    for (k, j, allasc) in stages:
                        if allasc:
                            lo = [(2 * j, W // (2 * j)), (1, j)]
                            for op, off in ((Alu.min, 0), (Alu.max, j)):
                                nc.vector.tensor_tensor(
                                    out=slot_ap(other, base + off, lo),
                                    in0=slot_ap(cur, base, lo),
                                    in1=slot_ap(cur, base + j, lo),
                                    op=op,
                                )
                        else:
                            dims = [(2 * k, W // (2 * k)), (2 * j, k // (2 * j)), (1, j)]
                            for desc in (0, 1):
                                b0 = base + (k if desc else 0)
                                lo_out, hi_out = (j, 0) if desc else (0, j)
                                nc.vector.tensor_tensor(
                                    out=slot_ap(other, b0 + lo_out, dims),
                                    in0=slot_ap(cur, b0, dims),
                                    in1=slot_ap(cur, b0 + j, dims),
                                    op=Alu.min,
                                )
                                nc.vector.tensor_tensor(
                                    out=slot_ap(other, b0 + hi_out, dims),
                                    in0=slot_ap(cur, b0, dims),
                                    in1=slot_ap(cur, b0 + j, dims),
                                    op=Alu.max,
                                )
                        cur, other = other, cur
                    return cur

                HRr = maxd - H                      # real R-half values
                HR = 0 if HRr < 1 else _next_pow2(HRr)
                if H >= 2 and HR == H:
                    # R needs the full depth: fused both-halves network
                    fin = emit_net(stages_for(H), 0, P2, v0, v1)
                    finR = fin
                else:
                    finR = (emit_net(stages_for(HR), H, HR, v0, v1)
                            if HRr >= 2 else v0)
                    fin = (emit_net(stages_for(H), 0, H, v0, v1)
                           if H >= 2 else v0)
                    if finR is not fin and HR > 0:
                        nc.vector.tensor_copy(
                            slot_ap(fin, H, [(1, HR)]),
                            slot_ap(finR, H, [(1, HR)]),
                        )

                # L sorted in fin[0:H], R sorted in fin[H:H+HR] (+inf beyond)
                o16 = mpool.tile([128, OUT_C], fp16, tag="o16")
                for ri, r in enumerate(rs):
                    m = mpool.tile([128, K // 2 + 1, OUT_C], fp16, tag=f"m{ri}")
                    sv = fin[:]
                    t0 = max(0, r - HR)
                    nc1 = r - t0                    # max(L[t], R[r-1-t]) cands
                    if nc1 > 0:
                        nc.vector.tensor_tensor(
                            out=slot_ap(m, 0, [(1, nc1)]),
                            in0=slot_ap(fin, t0, [(1, nc1)]),
                            in1=AP(sv.tensor, sv.offset + (H + r - 1 - t0) * S,
                                   [sv.ap[0], [-S, nc1], [1, OUT_C]]),
                            op=Alu.max,
                        )
                    if r < HR:
                        # cands += [L[r], R[r]]
                        nc.vector.tensor_copy(
                            slot_ap(m, nc1, [(1, 2)]),
                            AP(sv.tensor, sv.offset + r * S,
                               [sv.ap[0], [H * S, 2], [1, OUT_C]]),
                        )
                        ncand = nc1 + 2
                    else:
                        nc.vector.tensor_copy(
                            slot_ap(m, nc1, [(1, 1)]),
                            slot_ap(fin, r, [(1, 1)]),
                        )
                        ncand = nc1 + 1
                    # min-reduce cands[0..ncand) into cands[0]; the very last
                    # min writes o16 directly for the base rank
                    n = ncand
                    while n > 2:
                        a = n - n // 2
                        nc.vector.tensor_tensor(
                            out=slot_ap(m, 0, [(1, n // 2)]),
                            in0=slot_ap(m, 0, [(1, n // 2)]),
                            in1=slot_ap(m, a, [(1, n // 2)]),
                            op=Alu.min,
                        )
                        n = a
                    last_out = o16[:] if ri == 0 else slot_ap(m, 0, [(1, 1)])
                    if n == 2:
                        nc.vector.tensor_tensor(
                            out=last_out,
                            in0=slot_ap(m, 0, [(1, 1)]),
                            in1=slot_ap(m, 1, [(1, 1)]),
                            op=Alu.min,
                        )
                    elif ri == 0:
                        nc.vector.tensor_copy(o16[:], slot_ap(m, 0, [(1, 1)]))
                    if ri > 0:
                        pk = pick_sb[:]
                        pk_ap = AP(pk.tensor, pk.offset + kcol, [pk.ap[0], [0, OUT_C]])
                        kcol += 1
                        nc.vector.copy_predicated(o16[:], pk_ap, slot_ap(m, 0, [(1, 1)]))

                o32 = mpool.tile([128, OUT_C], fp32, tag="o32")
                nc.scalar.copy(o32[:], o16[:])
                nc.sync.dma_start(out[t * 128:(t + 1) * 128, :], o32[:])

    nc.compile()
    return nc


def _prepare(x, kernel, neighbors, degrees):
    """Host-side marshaling: permutation, schedule, idx/mask streams."""
    deg = np.clip(np.asarray(degrees).astype(np.int64), 1, K)
    deg_pad = np.ones(NPAD, np.int64)
    deg_pad[:N] = deg
    # dummies (N..NPAD) have deg 1 but gather only sentinels
    order = np.argsort(-deg_pad, kind="stable")        # global rank -> node id
    deg_sorted = deg_pad[order]
    sched = _make_schedule(deg_sorted)

    # table row of node u: rank j -> core j%8, local slot j//8; the table is
    # chunk-major (AGCH=7-tile pipelined AllGather chunks are contiguous):
    # row = chunk*(8*896) + core*896 + (local % 896)
    ranks = np.empty(NPAD, np.int64)
    ranks[order] = np.arange(NPAD)
    core = ranks % NCORES
    loc = ranks // NCORES
    CHR = 7 * 128                                      # rows/core/chunk
    rho = (loc // CHR) * (NCORES * CHR) + core * CHR + loc % CHR

    nbr = np.asarray(neighbors).astype(np.int64)
    nbr_rows = rho[nbr]                                # [N, K]
    pair_full = np.zeros((NPAD, K), np.int64)
    par_full = np.zeros((NPAD, K), np.int64)
    pair_full[:N] = nbr_rows >> 1
    par_full[:N] = nbr_rows & 1

    xf = np.zeros((NPAD, IN_C), np.float16)
    xf[:N] = np.asarray(x, np.float32).astype(np.float16)
    wf = np.asarray(kernel, np.float32).astype(np.float16)
    infs = np.full((4, OUT_C), np.inf, np.float16)

    karr = np.arange(K, dtype=np.int64)[None, :]

    in_maps = []
    node_of = np.empty((NCORES, SHARD), np.int64)
    for c in range(NCORES):
        nodes_c = order[c::NCORES]                     # local slot i -> node id
        node_of[c] = nodes_c
        d_c = deg_pad[nodes_c]                         # descending
        pair_c = pair_full[nodes_c]                    # [SHARD, K]
        par_c = par_full[nodes_c]
        valid_c = karr < d_c[:, None]                  # [SHARD, K]

        idx_parts = []
        par_parts = []
        pick_parts = []
        for t, (maxd, H, rs) in enumerate(sched):
            sl = slice(t * 128, (t + 1) * 128)
            pt = pair_c[sl, :maxd]                     # [128, maxd]
            vt = valid_c[sl, :maxd]
            stream = np.where(vt, pt, SENT_PAIR).T     # [maxd, 128] slot-major
            wrapped = np.tile(
                stream.reshape(maxd * 8, 16).T, (8, 1)
            )                                          # [128, maxd*8]
            idx_parts.append(wrapped.astype(np.int16))
            par_parts.append(
                np.where(vt, par_c[sl, :maxd], 0).astype(np.int16)
            )                                          # [128, maxd]
            r_t = (d_c[sl] - 1) // 2                   # [128]
            for r in rs[1:]:
                pick_parts.append((r_t == r).astype(np.int16)[:, None])

        idx_all = np.ascontiguousarray(np.concatenate(idx_parts, axis=1))
        par_all = np.ascontiguousarray(np.concatenate(par_parts, axis=1))
        if pick_parts:
            pick_all = np.ascontiguousarray(np.concatenate(pick_parts, axis=1))
        else:
            pick_all = np.zeros((128, 1), np.int16)
        in_maps.append({
            "xT": np.ascontiguousarray(xf[nodes_c].T),
            "w": wf,
            "idx": idx_all,
            "par": par_all,
            "pick": pick_all,
            "infs": infs,
        })

    return sched, in_maps, node_of


def kernel(x, kernel, neighbors, degrees):
    from concourse.bass_utils import run_bass_kernel_spmd

    sched, in_maps, node_of = _prepare(x, kernel, neighbors, degrees)
    if sched not in _CACHE:
        _CACHE[sched] = _emit_program(sched)
    nc = _CACHE[sched]

    res = run_bass_kernel_spmd(nc, in_maps, list(range(NCORES)))
    full = np.empty((NPAD, OUT_C), np.float32)
    for c in range(NCORES):
        full[node_of[c]] = res.results[c]["out"]
    return np.ascontiguousarray(full[:N])


# revision 28
# speedup vs baseline: 1.0749x; 1.0749x over previous
"""Median graph convolution on 8 Trainium2 NeuronCores.

out[n, c] = median over valid neighbors j of (x @ kernel)[neighbors[n, j], c]
(lower median, rank (deg-1)//2 of the first deg neighbor slots).

Strategy (data-parallel over nodes, 6272 nodes/core):
  - host sorts nodes by degree (descending), striped across the 8 cores so
    every core sees the same degree profile and one compiled program fits all
  - each core matmuls its node shard on the PE -> h shard (fp16),
    AllGather into a per-core HBM table with trailing +inf sentinel rows
  - the table is indexed as 512-byte PAIR rows (two h rows per descriptor),
    so the 50176-row table needs only 25089 int16-indexable pair rows;
    each real neighbor costs exactly ONE gather descriptor
  - only the first maxdeg(tile) slots are gathered per 128-node tile
    (pads ride as +inf sentinel descriptors / vector memset)
  - a copy + copy_predicated (int16 parity mask, stride-0 broadcast over
    channels) selects the wanted half of each gathered pair
  - a degree-adaptive bitonic network sorts the two H-halves of the slot
    array and a rank-r two-way merge formula extracts the lower median
"""

import sys

sys.path.insert(0, "/opt/trn_rl_repo")

import numpy as np

N, K, IN_C, OUT_C = 50000, 32, 256, 128
NCORES = 8
NTILES = 49                      # 128-node tiles per core
SHARD = NTILES * 128             # 6272
NPAD = SHARD * NCORES            # 50176
TROWS = NPAD + 4                 # +inf sentinel rows at the end
SENT_PAIR = NPAD // 2            # pair index of the +inf sentinel row pair
NPAIRS = SENT_PAIR + 1           # pair rows addressable by the gather
GCHUNK = 8                       # slots per dma_gather call (8*128 = 1024 idx)
MAXSLOTS = 32

_CACHE = {}


def _next_pow2(x):
    p = 1
    while p < x:
        p *= 2
    return p


def _make_schedule(deg_sorted):
    """Per-tile (maxd, H, r_list) from the global descending degree profile."""
    sched = []
    for t in range(NTILES):
        degs = deg_sorted[t * 128 * NCORES:(t + 1) * 128 * NCORES]
        maxd = int(degs[0])
        H = max(1, _next_pow2(maxd) // 2)
        rs = sorted({int((d - 1) // 2) for d in degs}, reverse=True)
        sched.append((maxd, H, tuple(rs)))
    return tuple(sched)


def _emit_program(sched):
    import concourse.tile as tile
    import concourse.mybir as mybir
    from concourse import bacc
    from concourse.bass import AP
    from concourse.library_config import mlp

    fp16 = mybir.dt.float16
    fp32 = mybir.dt.float32
    i16 = mybir.dt.int16
    Alu = mybir.AluOpType

    tot_idx_cols = sum(maxd * 8 for (maxd, _, _) in sched)
    tot_par_cols = sum(maxd for (maxd, _, _) in sched)
    tot_pick = sum(len(rs) - 1 for (_, _, rs) in sched)

    nc = bacc.Bacc("TRN2", target_bir_lowering=False, num_swdge_queues=4,
                   dynamic_dma_scratch_size=32768)

    xT = nc.dram_tensor("xT", [IN_C, SHARD], fp16, kind="ExternalInput")
    w = nc.dram_tensor("w", [IN_C, OUT_C], fp16, kind="ExternalInput")
    idx_d = nc.dram_tensor("idx", [128, tot_idx_cols], i16, kind="ExternalInput")
    par_d = nc.dram_tensor("par", [128, tot_par_cols], i16, kind="ExternalInput")
    pick_d = nc.dram_tensor("pick", [128, max(1, tot_pick)], i16, kind="ExternalInput")
    infs = nc.dram_tensor("infs", [4, OUT_C], fp16, kind="ExternalInput")  # +inf rows
    out = nc.dram_tensor("out", [SHARD, OUT_C], fp32, kind="ExternalOutput")
    table = nc.dram_tensor("table", [TROWS, OUT_C], fp16, addr_space="Shared")
    hshard = nc.dram_tensor("hshard", [SHARD, OUT_C], fp16)

    # gather source: the table viewed as 512B pair rows [NPAIRS, 256]
    pair_ap = AP(table[:].tensor, 0, [[2 * OUT_C, NPAIRS], [1, 2 * OUT_C]])

    S = OUT_C  # slot stride (elements) in the selected-value tile v

    def slot_ap(t, slot0, dims, stride=None):
        """AP over value tile t: partition dim + (slot_step, count) dims + c.

        stride overrides the slot stride in elements (256 for the raw pair
        buffer whose a-halves act as the stage-0 value array)."""
        ss = S if stride is None else stride
        base = t[:]
        free = [[st * ss, ct] for (st, ct) in dims if ct != 1]
        return AP(base.tensor, base.offset + slot0 * ss, [base.ap[0]] + free + [[1, OUT_C]])

    def stages_for(top):
        """(k, j, allasc) stage list; allasc on the final k group."""
        ks = []
        k = 2
        while k <= top:
            j = k // 2
            while j >= 1:
                ks.append((k, j, k == top))
                j //= 2
            k *= 2
        return ks

    with tile.TileContext(nc) as tc:
        nc.gpsimd.load_library(mlp)
        with (
            tc.tile_pool(name="const", bufs=1) as cpool,
            tc.tile_pool(name="psum", bufs=2, space="PSUM") as psum_pool,
            tc.tile_pool(name="gbuf", bufs=4) as gpool,
            tc.tile_pool(name="work", bufs=2) as wpool,
            tc.tile_pool(name="mout", bufs=2) as mpool,
        ):
            # ---- phase 1+2: h rows = x @ w (x chunk stationary -> [node, c]),
            # AllGather pipelined in chunks behind the matmul ----
            inft = cpool.tile([4, OUT_C], fp16)
            nc.sync.dma_start(inft[:], infs[:])
            nc.sync.dma_start(table[NPAD:NPAD + 4, :], inft[:])
            AGCH = 25                   # tiles in the first AllGather chunk
            with tc.tile_pool(name="stage", bufs=1) as spool:
                lw0 = spool.tile([128, OUT_C], fp16)
                lw1 = spool.tile([128, OUT_C], fp16)
                nc.sync.dma_start(lw0[:], w[0:128, :])
                nc.sync.dma_start(lw1[:], w[128:256, :])
                xt0 = spool.tile([128, SHARD], fp16)
                xt1 = spool.tile([128, SHARD], fp16)
                nc.sync.dma_start(xt0[:], xT[0:128, :])
                nc.sync.dma_start(xt1[:], xT[128:256, :])
                hrows = spool.tile([128, NTILES, OUT_C], fp16)
                tb = table[:]
                for j in range(NTILES):
                    ns = slice(j * 128, (j + 1) * 128)
                    ps = psum_pool.tile([128, OUT_C], fp32)
                    nc.tensor.matmul(ps[:], lhsT=xt0[:, ns], rhs=lw0[:], start=True, stop=False)
                    nc.tensor.matmul(ps[:], lhsT=xt1[:, ns], rhs=lw1[:], start=False, stop=True)
                    nc.scalar.copy(hrows[:, j, :], ps[:])
                    if j + 1 in (AGCH, NTILES):
                        c0 = 0 if j + 1 == AGCH else AGCH
                        nc.sync.dma_start(
                            hshard[c0 * 128:(j + 1) * 128, :].rearrange(
                                "(j n) c -> n j c", n=128),
                            hrows[:, c0:j + 1, :],
                        )
                        nc.gpsimd.collective_compute(
                            "AllGather",
                            mybir.AluOpType.bypass,
                            replica_groups=[list(range(NCORES))],
                            ins=[hshard[c0 * 128:(j + 1) * 128, :]],
                            outs=[table[c0 * 128 * NCORES:(j + 1) * 128 * NCORES, :]],
                        )

            # ---- load index/mask streams; +inf constant for pad slots ----
            idx_sb = cpool.tile([128, tot_idx_cols], i16)
            par_sb = cpool.tile([128, tot_par_cols], i16)
            pick_sb = cpool.tile([128, max(1, tot_pick)], i16)
            nc.sync.dma_start(idx_sb[:], idx_d[:])
            nc.sync.dma_start(par_sb[:], par_d[:])
            nc.sync.dma_start(pick_sb[:], pick_d[:])
            inf_const = cpool.tile([128, 15 * OUT_C], fp16)
            nc.vector.memset(inf_const[:], float("inf"))

            # ---- phase 3: gather + select + sort + median per tile ----
            icol = 0      # running idx column offset
            pcol = 0      # running parity column offset
            kcol = 0      # running pick-mask column offset
            qn = 0        # dma queue rotation
            for t, (maxd, H, rs) in enumerate(sched):
                P2 = 2 * H
                buf = gpool.tile([128, MAXSLOTS, 2 * OUT_C], fp16, tag="pair")
                for s0 in range(0, maxd, GCHUNK):
                    s1 = min(s0 + GCHUNK, maxd)
                    G = (s1 - s0) * 128
                    nc.gpsimd.dma_gather(
                        buf[:, s0:s1, :],
                        pair_ap,
                        idx_sb[:, icol + s0 * 8: icol + s1 * 8],
                        G, G, 2 * OUT_C,
                        queue_num=qn, single_packet=False)
                    qn = (qn + 1) % 4
                icol += maxd * 8

                # select the wanted half of each pair into v0: copy the
                # a-halves, overwrite with the b-half where the parity mask
                # is 1 (stride-0 broadcast mask), memset pad slots to +inf.
                SB = 2 * OUT_C
                bb = buf[:]
                a_ap = AP(bb.tensor, bb.offset, [bb.ap[0], [SB, maxd], [1, OUT_C]])
                b_ap = AP(bb.tensor, bb.offset + OUT_C, [bb.ap[0], [SB, maxd], [1, OUT_C]])
                pp = par_sb[:]
                m_ap = AP(pp.tensor, pp.offset + pcol, [pp.ap[0], [1, maxd], [0, OUT_C]])
                pcol += maxd

                v0 = wpool.tile([128, MAXSLOTS, OUT_C], fp16, tag="v0")
                v1 = wpool.tile([128, MAXSLOTS, OUT_C], fp16, tag="v1")
                # TT max(a,a) instead of tensor_copy: hits the 2x_1P packed
                # mode where the strided-source copy falls back to 1x
                nc.vector.tensor_tensor(
                    out=slot_ap(v0, 0, [(1, maxd)]), in0=a_ap, in1=a_ap, op=Alu.max)
                if maxd < P2:
                    nc.gpsimd.memset(slot_ap(v0, maxd, [(1, P2 - maxd)]), float("inf"))
                nc.vector.copy_predicated(slot_ap(v0, 0, [(1, maxd)]), m_ap, b_ap)

                def emit_net(stages, base, W, cur, other):
                    """Bitonic network over slots [base, base+W); returns the
                    tile holding the final values of that region."""
                    for (k, j, allasc) in stages:
                        if allasc:
                            lo = [(2 * j, W // (2 * j)), (1, j)]
                            for op, off in ((Alu.min, 0), (Alu.max, j)):
                                nc.vector.tensor_tensor(
                                    out=slot_ap(other, base + off, lo),
                                    in0=slot_ap(cur, base, lo),
                                    in1=slot_ap(cur, base + j, lo),
                                    op=op,
                                )
                        else:
                            dims = [(2 * k, W // (2 * k)), (2 * j, k // (2 * j)), (1, j)]
                            for desc in (0, 1):
                                b0 = base + (k if desc else 0)
                                lo_out, hi_out = (j, 0) if desc else (0, j)
                                nc.vector.tensor_tensor(
                                    out=slot_ap(other, b0 + lo_out, dims),
                                    in0=slot_ap(cur, b0, dims),
                                    in1=slot_ap(cur, b0 + j, dims),
                                    op=Alu.min,
                                )
                                nc.vector.tensor_tensor(
                                    out=slot_ap(other, b0 + hi_out, dims),
                                    in0=slot_ap(cur, b0, dims),
                                    in1=slot_ap(cur, b0 + j, dims),
                                    op=Alu.max,
                                )
                        cur, other = other, cur
                    return cur

                HRr = maxd - H                      # real R-half values
                HR = 0 if HRr < 1 else _next_pow2(HRr)
                if H >= 2 and HR == H:
                    # R needs the full depth: fused both-halves network
                    fin = emit_net(stages_for(H), 0, P2, v0, v1)
                    finR = fin
                else:
                    finR = (emit_net(stages_for(HR), H, HR, v0, v1)
                            if HRr >= 2 else v0)
                    fin = (emit_net(stages_for(H), 0, H, v0, v1)
                           if H >= 2 else v0)
                    if finR is not fin and HR > 0:
                        nc.vector.tensor_copy(
                            slot_ap(fin, H, [(1, HR)]),
                            slot_ap(finR, H, [(1, HR)]),
                        )

                # L sorted in fin[0:H], R sorted in fin[H:H+HR] (+inf beyond)
                o16 = mpool.tile([128, OUT_C], fp16, tag="o16")
                for ri, r in enumerate(rs):
                    m = mpool.tile([128, K // 2 + 1, OUT_C], fp16, tag=f"m{ri}")
                    sv = fin[:]
                    t0 = max(0, r - HR)
                    nc1 = r - t0                    # max(L[t], R[r-1-t]) cands
                    if nc1 > 0:
                        nc.vector.tensor_tensor(
                            out=slot_ap(m, 0, [(1, nc1)]),
                            in0=slot_ap(fin, t0, [(1, nc1)]),
                            in1=AP(sv.tensor, sv.offset + (H + r - 1 - t0) * S,
                                   [sv.ap[0], [-S, nc1], [1, OUT_C]]),
                            op=Alu.max,
                        )
                    if r < HR:
                        # cands += [L[r], R[r]]
                        nc.vector.tensor_copy(
                            slot_ap(m, nc1, [(1, 2)]),
                            AP(sv.tensor, sv.offset + r * S,
                               [sv.ap[0], [H * S, 2], [1, OUT_C]]),
                        )
                        ncand = nc1 + 2
                    else:
                        nc.vector.tensor_copy(
                            slot_ap(m, nc1, [(1, 1)]),
                            slot_ap(fin, r, [(1, 1)]),
                        )
                        ncand = nc1 + 1
                    # min-reduce cands[0..ncand) into cands[0]; the very last
                    # min writes o16 directly for the base rank
                    n = ncand
                    while n > 2:
                        a = n - n // 2
                        nc.vector.tensor_tensor(
                            out=slot_ap(m, 0, [(1, n // 2)]),
                            in0=slot_ap(m, 0, [(1, n // 2)]),
                            in1=slot_ap(m, a, [(1, n // 2)]),
                            op=Alu.min,
                        )
                        n = a
                    last_out = o16[:] if ri == 0 else slot_ap(m, 0, [(1, 1)])
                    if n == 2:
                        nc.vector.tensor_tensor(
                            out=last_out,
                            in0=slot_ap(m, 0, [(1, 1)]),
                            in1=slot_ap(m, 1, [(1, 1)]),
                            op=Alu.min,
                        )
                    elif ri == 0:
                        nc.vector.tensor_copy(o16[:], slot_ap(m, 0, [(1, 1)]))
                    if ri > 0:
                        pk = pick_sb[:]
                        pk_ap = AP(pk.tensor, pk.offset + kcol, [pk.ap[0], [0, OUT_C]])
                        kcol += 1
                        nc.vector.copy_predicated(o16[:], pk_ap, slot_ap(m, 0, [(1, 1)]))

                o32 = mpool.tile([128, OUT_C], fp32, tag="o32")
                nc.scalar.copy(o32[:], o16[:])
                nc.sync.dma_start(out[t * 128:(t + 1) * 128, :], o32[:])

    nc.compile()
    return nc


def _prepare(x, kernel, neighbors, degrees):
    """Host-side marshaling: permutation, schedule, idx/mask streams."""
    deg = np.clip(np.asarray(degrees).astype(np.int64), 1, K)
    deg_pad = np.ones(NPAD, np.int64)
    deg_pad[:N] = deg
    # dummies (N..NPAD) have deg 1 but gather only sentinels
    order = np.argsort(-deg_pad, kind="stable")        # global rank -> node id
    deg_sorted = deg_pad[order]
    sched = _make_schedule(deg_sorted)

    # table row of node u: rank j -> core j%8, local slot j//8; the table is
    # chunk-major (AGCH=7-tile pipelined AllGather chunks are contiguous):
    # row = chunk*(8*896) + core*896 + (local % 896)
    ranks = np.empty(NPAD, np.int64)
    ranks[order] = np.arange(NPAD)
    core = ranks % NCORES
    loc = ranks // NCORES
    CHR0 = 25 * 128                                    # chunk-0 rows/core
    rho = np.where(
        loc < CHR0,
        core * CHR0 + loc,
        NCORES * CHR0 + core * (SHARD - CHR0) + (loc - CHR0),
    )

    nbr = np.asarray(neighbors).astype(np.int64)
    nbr_rows = rho[nbr]                                # [N, K]
    pair_full = np.zeros((NPAD, K), np.int64)
    par_full = np.zeros((NPAD, K), np.int64)
    pair_full[:N] = nbr_rows >> 1
    par_full[:N] = nbr_rows & 1

    xf = np.zeros((NPAD, IN_C), np.float16)
    xf[:N] = np.asarray(x, np.float32).astype(np.float16)
    wf = np.asarray(kernel, np.float32).astype(np.float16)
    infs = np.full((4, OUT_C), np.inf, np.float16)

    karr = np.arange(K, dtype=np.int64)[None, :]

    in_maps = []
    node_of = np.empty((NCORES, SHARD), np.int64)
    for c in range(NCORES):
        nodes_c = order[c::NCORES]                     # local slot i -> node id
        node_of[c] = nodes_c
        d_c = deg_pad[nodes_c]                         # descending
        pair_c = pair_full[nodes_c]                    # [SHARD, K]
        par_c = par_full[nodes_c]
        valid_c = karr < d_c[:, None]                  # [SHARD, K]

        idx_parts = []
        par_parts = []
        pick_parts = []
        for t, (maxd, H, rs) in enumerate(sched):
            sl = slice(t * 128, (t + 1) * 128)
            pt = pair_c[sl, :maxd]                     # [128, maxd]
            vt = valid_c[sl, :maxd]
            stream = np.where(vt, pt, SENT_PAIR).T     # [maxd, 128] slot-major
            wrapped = np.tile(
                stream.reshape(maxd * 8, 16).T, (8, 1)
            )                                          # [128, maxd*8]
            idx_parts.append(wrapped.astype(np.int16))
            par_parts.append(
                np.where(vt, par_c[sl, :maxd], 0).astype(np.int16)
            )                                          # [128, maxd]
            r_t = (d_c[sl] - 1) // 2                   # [128]
            for r in rs[1:]:
                pick_parts.append((r_t == r).astype(np.int16)[:, None])

        idx_all = np.ascontiguousarray(np.concatenate(idx_parts, axis=1))
        par_all = np.ascontiguousarray(np.concatenate(par_parts, axis=1))
        if pick_parts:
            pick_all = np.ascontiguousarray(np.concatenate(pick_parts, axis=1))
        else:
            pick_all = np.zeros((128, 1), np.int16)
        in_maps.append({
            "xT": np.ascontiguousarray(xf[nodes_c].T),
            "w": wf,
            "idx": idx_all,
            "par": par_all,
            "pick": pick_all,
            "infs": infs,
        })

    return sched, in_maps, node_of


def kernel(x, kernel, neighbors, degrees):
    from concourse.bass_utils import run_bass_kernel_spmd

    sched, in_maps, node_of = _prepare(x, kernel, neighbors, degrees)
    if sched not in _CACHE:
        _CACHE[sched] = _emit_program(sched)
    nc = _CACHE[sched]

    res = run_bass_kernel_spmd(nc, in_maps, list(range(NCORES)))
    full = np.empty((NPAD, OUT_C), np.float32)
    for c in range(NCORES):
        full[node_of[c]] = res.results[c]["out"]
    return np.ascontiguousarray(full[:N])
